# revision 11
# baseline (speedup 1.0000x reference)
"""Trainium2 Bass kernel for nn_NoncommutativeKATRepresentation.

Math: out[b,q] = T_q(xm_b) * c_q * s[b,q] / 100 for q=0..99, where
  s[b,q]   = sum_p sin(pi*(q+1)*x[b,p]) * W[q,p]
  c_q      = coeff_q * exp(-0.03*(q+1)^2)            (f32; == 0 for q >= 58)
  xm_b     = mean_q tanh(c_q * s[b,q])
  T_q      = Chebyshev polynomial = cos(q * arccos(xm))
The theta/star-product correction in the reference is exactly zero.
Columns q >= 58 are exactly zero in f32 (exp underflow), so the device
computes only 58 harmonics.

Sharding: pure data parallel over the batch dim across 8 cores (512 rows
each). x is passed pre-transposed per core as [100, 512] (p on partitions,
batch on the free dim). The contraction over p runs on the tensor engine
as 58 accumulating matmuls with one-hot-weighted stationaries.

HW notes that shaped this implementation:
 - ACT `Sin` has NO range reduction: args must be within [-pi, pi].
   Per k we compute u = (k*x) mod 2 in one fused DVE tensor_scalar op and
   evaluate sin(pi*u - pi) = -sin(pi*k*x); the sign is folded into
   negated W stationaries.
 - fp32 matmul costs 4 cycles/row, bf16 1 cycle/row. k <= 15 uses fp32;
   k > 15 uses bf16 (those columns are <= 3e-5 of the output scale, and
   their x_mean contribution is < 1e-7).
 - T_q via closed form cos(q*arccos(xm)) (|xm| <= 0.06): q*theta built by
   a rank-1 matmul, range-reduced with the triangle-wave identity
   cos(pi*p) = cos(pi*|((p+1) mod 2) - 1|).
"""

import numpy as np
import ml_dtypes

import concourse.bacc as bacc
import concourse.tile as tile
import concourse.mybir as mybir
from concourse.bass_utils import run_bass_kernel_spmd

F32 = mybir.dt.float32
BF16 = mybir.dt.bfloat16
AF = mybir.ActivationFunctionType
OP = mybir.AluOpType

N_CORES = 8
B_FULL = 4096
IN = 100
D_OUT = 100
BL = B_FULL // N_CORES          # 512 batch rows per core
KMAX = 58                       # harmonics with nonzero f32 exp term
K_EXACT = 15                    # k <= K_EXACT contracted in fp32
NQ = KMAX                       # output columns computed on device
PI = float(np.pi)

_PROG = None  # cached compiled Bass program


def _coeffs():
    """c_q = coeff_q * exp_term_q computed in f32, mimicking the reference."""
    q = np.arange(KMAX, dtype=np.float32)
    k = (q + np.float32(1.0)).astype(np.float32)
    with np.errstate(divide="ignore", invalid="ignore"):
        s = np.float32(1.0) + np.float32(0.2) * np.log(
            q / np.float32(15.0), dtype=np.float32
        ) * (
            np.float32(1.0)
            - np.exp(np.float32(-0.03) * (q - np.float32(15.0)), dtype=np.float32)
        )
    coeff = np.where(q < 15, np.float32(1.0) / k, np.float32(1.0) / (k * s))
    coeff = coeff.astype(np.float32)
    exp_term = np.exp(np.float32(-0.03) * k * k, dtype=np.float32)
    return (coeff * exp_term).astype(np.float32)


def _build_program():
    nc = bacc.Bacc("TRN2", target_bir_lowering=False, debug=False,
                   num_devices=N_CORES)

    xt = nc.dram_tensor("xt", [IN, BL], F32, kind="ExternalInput")
    wstat_f = nc.dram_tensor("wstat_f", [IN, K_EXACT * NQ], F32,
                             kind="ExternalInput")
    wstat_h = nc.dram_tensor("wstat_h", [IN, (KMAX - K_EXACT) * NQ], BF16,
                             kind="ExternalInput")
    cons = nc.dram_tensor("cons", [128, 8], F32, kind="ExternalInput")
    qpi = nc.dram_tensor("qpi", [1, NQ], F32, kind="ExternalInput")
    ident = nc.dram_tensor("ident", [128, 128], F32, kind="ExternalInput")
    out_qb = nc.dram_tensor("out_qb", [NQ, BL], F32, kind="ExternalOutput")

    with tile.TileContext(nc) as tc:
        with (
            tc.tile_pool(name="const", bufs=1) as cpool,
            tc.tile_pool(name="upool", bufs=6) as upool,
            tc.tile_pool(name="sinf", bufs=4) as sinf_pool,
            tc.tile_pool(name="sinh", bufs=6) as sinh_pool,
            tc.tile_pool(name="epi", bufs=1) as epi,
            tc.tile_pool(name="small", bufs=1) as small,
            tc.tile_pool(name="ps", bufs=1, space="PSUM") as ps,
        ):
            t_xt = cpool.tile([IN, BL], F32)
            t_wf = cpool.tile([IN, K_EXACT * NQ], F32)
            t_wh = cpool.tile([IN, (KMAX - K_EXACT) * NQ], BF16)
            t_cons = cpool.tile([128, 8], F32)
            t_qpi = cpool.tile([1, NQ], F32)
            t_id = cpool.tile([128, 128], F32)
            nc.sync.dma_start(t_xt[:], xt[:])
            nc.sync.dma_start(t_wf[:], wstat_f[:])
            nc.sync.dma_start(t_wh[:], wstat_h[:])
            nc.sync.dma_start(t_cons[:], cons[:])
            nc.sync.dma_start(t_qpi[:], qpi[:])
            nc.sync.dma_start(t_id[:], ident[:])

            # ---- main loop: s[q, b] accumulated in PSUM ----
            # v = (k/2)*x - rint((k/2)*x)  in [-0.5, 0.5];
            # sin(pi*k*x) = sin(2*pi*v).  rint via the 1.5*2^23 magic pair.
            MAGIC = 12582912.0
            S_ps = ps.tile([NQ, BL], F32)
            for i in range(KMAX):
                k = i + 1
                if k <= K_EXACT:
                    sk = sinf_pool.tile([IN, BL], F32, tag="sf")
                    stat = t_wf[:, i * NQ:(i + 1) * NQ]
                else:
                    sk = sinh_pool.tile([IN, BL], BF16, tag="sh")
                    stat = t_wh[:, (i - K_EXACT) * NQ:(i - K_EXACT + 1) * NQ]
                if k == 1:
                    # pi*x already in [0, pi)
                    nc.scalar.activation(sk[:], t_xt[:], AF.Sin, scale=PI)
                else:
                    h = upool.tile([IN, BL], F32, tag="h")
                    nc.vector.tensor_scalar(h[:], t_xt[:], 0.5 * k, None,
                                            OP.mult)
                    rh = upool.tile([IN, BL], F32, tag="rh")
                    nc.vector.tensor_scalar(rh[:], h[:], MAGIC, MAGIC,
                                            OP.add, OP.subtract)
                    v = upool.tile([IN, BL], F32, tag="v")
                    # spread the subtract across DVE and GpSimd
                    if k % 3 != 0:
                        nc.gpsimd.tensor_sub(v[:], h[:], rh[:])
                    else:
                        nc.vector.tensor_sub(v[:], h[:], rh[:])
                    nc.scalar.activation(sk[:], v[:], AF.Sin, scale=2.0 * PI)
                nc.tensor.matmul(S_ps[:], stat, sk[:],
                                 start=(i == 0), stop=(i == KMAX - 1))

            # ---- epilogue ----
            # xn = tanh(c_q * s); Sc = (c_q/100) * s
            t_xn = epi.tile([NQ, BL], F32)
            nc.scalar.activation(t_xn[:], S_ps[:], AF.Tanh,
                                 scale=t_cons[:NQ, 0:1])
            t_sc = epi.tile([NQ, BL], F32)
            nc.scalar.activation(t_sc[:], S_ps[:], AF.Copy,
                                 scale=t_cons[:NQ, 1:2])

            # x_mean directly in [128, 4] layout: one matmul per 128-batch
            # chunk with xn-chunk as the stationary operand.
            xm_ps = ps.tile([128, 4], F32)
            for j in range(4):
                nc.tensor.matmul(xm_ps[:, j:j + 1],
                                 t_xn[:, j * 128:(j + 1) * 128],
                                 t_cons[:NQ, 2:3], start=True, stop=True)

            t_m = small.tile([128, 4], F32)
            nc.vector.tensor_copy(t_m[:], xm_ps[:])
            t_mc = small.tile([128, 4], F32)
            nc.vector.tensor_scalar(t_mc[:], t_m[:], 0.5, -0.5, OP.min, OP.max)
            # theta0 = pi/2 - m - m^3/6
            t_m2 = small.tile([128, 4], F32)
            nc.vector.tensor_mul(t_m2[:], t_mc[:], t_mc[:])
            t_m3 = small.tile([128, 4], F32)
            nc.vector.tensor_mul(t_m3[:], t_m2[:], t_mc[:])
            t_a = small.tile([128, 4], F32)
            nc.vector.tensor_scalar(t_a[:], t_m3[:], -1.0 / 6.0, PI / 2.0,
                                    OP.mult, OP.add)
            t_th = small.tile([128, 4], F32)
            nc.vector.tensor_sub(t_th[:], t_a[:], t_mc[:])
            # one Newton step: theta += (cos th - m) / sin th
            t_sth = small.tile([128, 4], F32)
            nc.scalar.activation(t_sth[:], t_th[:], AF.Sin)
            t_cth = small.tile([128, 4], F32)
            nc.scalar.activation(t_cth[:], t_th[:], AF.Sin,
                                 bias=t_cons[:, 4:5])
            t_r = small.tile([128, 4], F32)
            nc.vector.reciprocal(t_r[:], t_sth[:])
            t_d = small.tile([128, 4], F32)
            nc.vector.tensor_sub(t_d[:], t_cth[:], t_mc[:])
            t_e = small.tile([128, 4], F32)
            nc.vector.tensor_mul(t_e[:], t_d[:], t_r[:])
            t_th2 = small.tile([128, 4], F32)
            nc.vector.tensor_add(t_th2[:], t_th[:], t_e[:])

            # theta as 4 rows of [1, 128] via per-column PE transposes
            t_throws = []
            for j in range(4):
                tp_ps = ps.tile([1, 128], F32, tag="tp")
                nc.tensor.transpose(tp_ps[:], t_th2[:, j:j + 1], t_id[:])
                tr = small.tile([1, 128], F32, tag=f"thr{j}")
                nc.vector.tensor_copy(tr[:], tp_ps[:])
                t_throws.append(tr)

            # phase[q, b] = (q/pi) * theta_b ; 4 rank-1 matmuls
            ph_ps = ps.tile([NQ, BL], F32)
            for j in range(4):
                nc.tensor.matmul(ph_ps[:, j * 128:(j + 1) * 128],
                                 t_qpi[:], t_throws[j][0:1, :],
                                 start=True, stop=True)
            # ph = q*theta/(2*pi).  T_q = cos(2*pi*ph) = -sin(2*pi*u) with
            # u = (ph - 1/4) - rint(ph - 1/4); the -1 is folded into cons[:,1].
            t_e = epi.tile([NQ, BL], F32)
            nc.vector.tensor_scalar(t_e[:], ph_ps[:], -0.25, None, OP.add)
            t_re = epi.tile([NQ, BL], F32)
            nc.vector.tensor_scalar(t_re[:], t_e[:], MAGIC, MAGIC,
                                    OP.add, OP.subtract)
            t_u = epi.tile([NQ, BL], F32)
            nc.vector.tensor_sub(t_u[:], t_e[:], t_re[:])
            t_outer = epi.tile([NQ, BL], F32)
            nc.scalar.activation(t_outer[:], t_u[:], AF.Sin, scale=2.0 * PI)
            t_res = epi.tile([NQ, BL], F32)
            nc.vector.tensor_mul(t_res[:], t_outer[:], t_sc[:])
            nc.sync.dma_start(out_qb[:], t_res[:])

    nc.compile()
    return nc


def _prepare_static_inputs():
    """Inputs that do not depend on x (built once)."""
    return None


def _host_inputs(x, W):
    c = _coeffs()
    negW = (W[:KMAX, :IN]).astype(np.float32)       # [58, 100]

    wf = np.zeros((IN, K_EXACT * NQ), dtype=np.float32)
    for i in range(K_EXACT):
        wf[:, i * NQ + i] = negW[i, :]
    wh = np.zeros((IN, (KMAX - K_EXACT) * NQ), dtype=np.float32)
    for i in range(K_EXACT, KMAX):
        j = i - K_EXACT
        wh[:, j * NQ + i] = negW[i, :]
    wh = wh.astype(ml_dtypes.bfloat16)

    cons = np.zeros((128, 8), dtype=np.float32)
    cons[:NQ, 0] = c
    cons[:NQ, 1] = -c * np.float32(0.01)
    cons[:NQ, 2] = np.float32(0.01)
    cons[:, 3] = np.float32(-np.pi)
    cons[:, 4] = np.float32(np.pi / 2)
    qp = (np.arange(NQ, dtype=np.float64) / (2 * np.pi)).astype(np.float32).reshape(1, NQ)
    idm = np.eye(128, dtype=np.float32)
    shared = dict(wstat_f=wf, wstat_h=wh, cons=cons, qpi=qp, ident=idm)
    maps = []
    for ci in range(N_CORES):
        xs = x[ci * BL:(ci + 1) * BL, :]            # [512, 100]
        xtc = np.ascontiguousarray(xs.T.astype(np.float32))  # [100, 512]
        maps.append(dict(shared, xt=xtc))
    return maps


def _run(x, W, trace=False, trace_kwargs=None):
    global _PROG
    if _PROG is None:
        _PROG = _build_program()
    maps = _host_inputs(x, W)
    res = run_bass_kernel_spmd(_PROG, maps, list(range(N_CORES)),
                               trace=trace, **(trace_kwargs or {}))
    out = np.zeros((B_FULL, D_OUT), dtype=np.float32)
    for ci in range(N_CORES):
        out[ci * BL:(ci + 1) * BL, :NQ] = res.results[ci]["out_qb"].T
    return out, res


def kernel(x, inner_coefficients, theta_matrix, dimension):
    x = np.asarray(x, dtype=np.float32)
    W = np.asarray(inner_coefficients, dtype=np.float32)
    out, _ = _run(x, W, trace=False)
    return out


# revision 17
# speedup vs baseline: 1.0532x; 1.0532x over previous
"""Trainium2 Bass kernel for nn_NoncommutativeKATRepresentation.

Math: out[b,q] = T_q(xm_b) * c_q * s[b,q] / 100 for q=0..99, where
  s[b,q]   = sum_p sin(pi*(q+1)*x[b,p]) * W[q,p]
  c_q      = coeff_q * exp(-0.03*(q+1)^2)            (f32; == 0 for q >= 58)
  xm_b     = mean_q tanh(c_q * s[b,q])
  T_q      = Chebyshev polynomial = cos(q * arccos(xm))
The theta/star-product correction in the reference is exactly zero.
Columns q >= 58 are exactly zero in f32 (exp underflow), so the device
computes only 58 harmonics.

Sharding: pure data parallel over the batch dim across 8 cores (512 rows
each). x is passed pre-transposed per core as [100, 512] (p on partitions,
batch on the free dim). The contraction over p runs on the tensor engine
as 58 accumulating matmuls with one-hot-weighted stationaries.

HW notes that shaped this implementation:
 - ACT `Sin` has NO range reduction: args must be within [-pi, pi].
   Per k we compute u = (k*x) mod 2 in one fused DVE tensor_scalar op and
   evaluate sin(pi*u - pi) = -sin(pi*k*x); the sign is folded into
   negated W stationaries.
 - fp32 matmul costs 4 cycles/row, bf16 1 cycle/row. k <= 15 uses fp32;
   k > 15 uses bf16 (those columns are <= 3e-5 of the output scale, and
   their x_mean contribution is < 1e-7).
 - T_q via closed form cos(q*arccos(xm)) (|xm| <= 0.06): q*theta built by
   a rank-1 matmul, range-reduced with the triangle-wave identity
   cos(pi*p) = cos(pi*|((p+1) mod 2) - 1|).
"""

import numpy as np
import ml_dtypes

import concourse.bacc as bacc
import concourse.tile as tile
import concourse.mybir as mybir
from concourse.bass_utils import run_bass_kernel_spmd

F32 = mybir.dt.float32
BF16 = mybir.dt.bfloat16
AF = mybir.ActivationFunctionType
OP = mybir.AluOpType

N_CORES = 8
B_FULL = 4096
IN = 100
D_OUT = 100
BL = B_FULL // N_CORES          # 512 batch rows per core
KMAX = 58                       # harmonics with nonzero f32 exp term
K_EXACT = 15                    # k <= K_EXACT contracted in fp32
NQ = KMAX                       # output columns computed on device
PI = float(np.pi)

_PROG = None  # cached compiled Bass program


def _coeffs():
    """c_q = coeff_q * exp_term_q computed in f32, mimicking the reference."""
    q = np.arange(KMAX, dtype=np.float32)
    k = (q + np.float32(1.0)).astype(np.float32)
    with np.errstate(divide="ignore", invalid="ignore"):
        s = np.float32(1.0) + np.float32(0.2) * np.log(
            q / np.float32(15.0), dtype=np.float32
        ) * (
            np.float32(1.0)
            - np.exp(np.float32(-0.03) * (q - np.float32(15.0)), dtype=np.float32)
        )
    coeff = np.where(q < 15, np.float32(1.0) / k, np.float32(1.0) / (k * s))
    coeff = coeff.astype(np.float32)
    exp_term = np.exp(np.float32(-0.03) * k * k, dtype=np.float32)
    return (coeff * exp_term).astype(np.float32)


def _build_program():
    nc = bacc.Bacc("TRN2", target_bir_lowering=False, debug=False,
                   num_devices=N_CORES)

    xt = nc.dram_tensor("xt", [IN, BL], F32, kind="ExternalInput")
    wstat_f = nc.dram_tensor("wstat_f", [IN, K_EXACT * NQ], F32,
                             kind="ExternalInput")
    wstat_h = nc.dram_tensor("wstat_h", [IN, (KMAX - K_EXACT) * NQ], BF16,
                             kind="ExternalInput")
    cons = nc.dram_tensor("cons", [128, 8], F32, kind="ExternalInput")
    qpi = nc.dram_tensor("qpi", [1, NQ], F32, kind="ExternalInput")
    ident = nc.dram_tensor("ident", [128, 128], F32, kind="ExternalInput")
    out_qb = nc.dram_tensor("out_qb", [NQ, BL], F32, kind="ExternalOutput")

    with tile.TileContext(nc) as tc:
        with (
            tc.tile_pool(name="const", bufs=1) as cpool,
            tc.tile_pool(name="upool", bufs=4) as upool,
            tc.tile_pool(name="sinf", bufs=3) as sinf_pool,
            tc.tile_pool(name="sinh", bufs=3) as sinh_pool,
            tc.tile_pool(name="epi", bufs=1) as epi,
            tc.tile_pool(name="small", bufs=1) as small,
            tc.tile_pool(name="ps", bufs=1, space="PSUM") as ps,
        ):
            t_xt = cpool.tile([IN, BL], F32)
            t_wf = cpool.tile([IN, K_EXACT * NQ], F32)
            t_wh = cpool.tile([IN, (KMAX - K_EXACT) * NQ], BF16)
            t_cons = cpool.tile([128, 8], F32)
            t_qpi = cpool.tile([1, NQ], F32)
            t_id = cpool.tile([128, 128], F32)
            nc.sync.dma_start(t_xt[:], xt[:])
            nc.sync.dma_start(t_wf[:], wstat_f[:])
            nc.sync.dma_start(t_wh[:], wstat_h[:])
            nc.sync.dma_start(t_cons[:], cons[:])
            nc.sync.dma_start(t_qpi[:], qpi[:])
            nc.sync.dma_start(t_id[:], ident[:])

            # ---- main loop: s[q, b] accumulated in PSUM ----
            # v = (k/2)*x - rint((k/2)*x)  in [-0.5, 0.5];
            # sin(pi*k*x) = sin(2*pi*v).  rint via the 1.5*2^23 magic pair.
            # The rint / subtract / Sin ops are fused across groups of up
            # to 4 harmonics (identical scale 2*pi) to amortize per-op
            # fixed overheads; subtracts alternate between GpSimd and DVE.
            MAGIC = 12582912.0
            S_ps = ps.tile([NQ, BL], F32)

            def mm(i, sbuf_slice):
                k = i + 1
                if k <= K_EXACT:
                    stat = t_wf[:, i * NQ:(i + 1) * NQ]
                else:
                    stat = t_wh[:, (i - K_EXACT) * NQ:(i - K_EXACT + 1) * NQ]
                nc.tensor.matmul(S_ps[:], stat, sbuf_slice,
                                 start=(i == 0), stop=(i == KMAX - 1))

            # k = 1 directly: pi*x in [0, pi)
            sk1 = sinf_pool.tile([IN, BL], F32, tag="sf1")
            nc.scalar.activation(sk1[:], t_xt[:], AF.Sin, scale=PI)
            mm(0, sk1[:])

            groups = []
            ks = list(range(2, K_EXACT + 1))
            groups += [ks[j:j + 4] for j in range(0, len(ks), 4)]
            ks = list(range(K_EXACT + 1, KMAX + 1))
            groups += [ks[j:j + 4] for j in range(0, len(ks), 4)]
            for gi, grp in enumerate(groups):
                g = len(grp)
                hb = upool.tile([IN, g * BL], F32, tag="h")
                for j, k in enumerate(grp):
                    nc.vector.tensor_scalar(hb[:, j * BL:(j + 1) * BL],
                                            t_xt[:], 0.5 * k, None, OP.mult)
                rb = upool.tile([IN, g * BL], F32, tag="rh")
                nc.vector.tensor_scalar(rb[:], hb[:], MAGIC, MAGIC,
                                        OP.add, OP.subtract)
                vb = upool.tile([IN, g * BL], F32, tag="v")
                if gi % 3 != 0:
                    nc.gpsimd.tensor_sub(vb[:], hb[:], rb[:])
                else:
                    nc.vector.tensor_sub(vb[:], hb[:], rb[:])
                if grp[0] <= K_EXACT:
                    sb = sinf_pool.tile([IN, g * BL], F32, tag="sf")
                else:
                    sb = sinh_pool.tile([IN, g * BL], BF16, tag="sh")
                nc.scalar.activation(sb[:], vb[:], AF.Sin, scale=2.0 * PI)
                for j, k in enumerate(grp):
                    mm(k - 1, sb[:, j * BL:(j + 1) * BL])

            # ---- epilogue ----
            # xn = tanh(c_q * s); Sc = (c_q/100) * s
            t_xn = epi.tile([NQ, BL], F32)
            nc.scalar.activation(t_xn[:], S_ps[:], AF.Tanh,
                                 scale=t_cons[:NQ, 0:1])
            t_sc = epi.tile([NQ, BL], F32)
            nc.scalar.activation(t_sc[:], S_ps[:], AF.Copy,
                                 scale=t_cons[:NQ, 1:2])

            # x_mean directly in [128, 4] layout: one matmul per 128-batch
            # chunk with xn-chunk as the stationary operand.
            xm_ps = ps.tile([128, 4], F32)
            for j in range(4):
                nc.tensor.matmul(xm_ps[:, j:j + 1],
                                 t_xn[:, j * 128:(j + 1) * 128],
                                 t_cons[:NQ, 2:3], start=True, stop=True)

            t_m = small.tile([128, 4], F32)
            nc.vector.tensor_copy(t_m[:], xm_ps[:])
            t_mc = small.tile([128, 4], F32)
            nc.vector.tensor_scalar(t_mc[:], t_m[:], 0.5, -0.5, OP.min, OP.max)
            # theta0 = pi/2 - m - m^3/6
            t_m2 = small.tile([128, 4], F32)
            nc.vector.tensor_mul(t_m2[:], t_mc[:], t_mc[:])
            t_m3 = small.tile([128, 4], F32)
            nc.vector.tensor_mul(t_m3[:], t_m2[:], t_mc[:])
            t_a = small.tile([128, 4], F32)
            nc.vector.tensor_scalar(t_a[:], t_m3[:], -1.0 / 6.0, PI / 2.0,
                                    OP.mult, OP.add)
            t_th = small.tile([128, 4], F32)
            nc.vector.tensor_sub(t_th[:], t_a[:], t_mc[:])
            # one Newton step: theta += (cos th - m) / sin th
            t_sth = small.tile([128, 4], F32)
            nc.scalar.activation(t_sth[:], t_th[:], AF.Sin)
            t_cth = small.tile([128, 4], F32)
            nc.scalar.activation(t_cth[:], t_th[:], AF.Sin,
                                 bias=t_cons[:, 4:5])
            t_r = small.tile([128, 4], F32)
            nc.vector.reciprocal(t_r[:], t_sth[:])
            t_d = small.tile([128, 4], F32)
            nc.vector.tensor_sub(t_d[:], t_cth[:], t_mc[:])
            t_e = small.tile([128, 4], F32)
            nc.vector.tensor_mul(t_e[:], t_d[:], t_r[:])
            t_th2 = small.tile([128, 4], F32)
            nc.vector.tensor_add(t_th2[:], t_th[:], t_e[:])

            # theta as 4 rows of [1, 128] via per-column PE transposes
            t_throws = []
            for j in range(4):
                tp_ps = ps.tile([1, 128], F32, tag="tp")
                nc.tensor.transpose(tp_ps[:], t_th2[:, j:j + 1], t_id[:])
                tr = small.tile([1, 128], F32, tag=f"thr{j}")
                nc.vector.tensor_copy(tr[:], tp_ps[:])
                t_throws.append(tr)

            # phase[q, b] = (q/pi) * theta_b ; 4 rank-1 matmuls
            ph_ps = ps.tile([NQ, BL], F32)
            for j in range(4):
                nc.tensor.matmul(ph_ps[:, j * 128:(j + 1) * 128],
                                 t_qpi[:], t_throws[j][0:1, :],
                                 start=True, stop=True)
            # ph = q*theta/(2*pi).  T_q = cos(2*pi*ph) = -sin(2*pi*u) with
            # u = (ph - 1/4) - rint(ph - 1/4); the -1 is folded into cons[:,1].
            t_e = epi.tile([NQ, BL], F32)
            nc.vector.tensor_scalar(t_e[:], ph_ps[:], -0.25, None, OP.add)
            t_re = epi.tile([NQ, BL], F32)
            nc.vector.tensor_scalar(t_re[:], t_e[:], MAGIC, MAGIC,
                                    OP.add, OP.subtract)
            t_u = epi.tile([NQ, BL], F32)
            nc.vector.tensor_sub(t_u[:], t_e[:], t_re[:])
            t_outer = epi.tile([NQ, BL], F32)
            nc.scalar.activation(t_outer[:], t_u[:], AF.Sin, scale=2.0 * PI)
            t_res = epi.tile([NQ, BL], F32)
            nc.vector.tensor_mul(t_res[:], t_outer[:], t_sc[:])
            nc.sync.dma_start(out_qb[:], t_res[:])

    nc.compile()
    return nc


def _prepare_static_inputs():
    """Inputs that do not depend on x (built once)."""
    return None


def _host_inputs(x, W):
    c = _coeffs()
    negW = (W[:KMAX, :IN]).astype(np.float32)       # [58, 100]

    wf = np.zeros((IN, K_EXACT * NQ), dtype=np.float32)
    for i in range(K_EXACT):
        wf[:, i * NQ + i] = negW[i, :]
    wh = np.zeros((IN, (KMAX - K_EXACT) * NQ), dtype=np.float32)
    for i in range(K_EXACT, KMAX):
        j = i - K_EXACT
        wh[:, j * NQ + i] = negW[i, :]
    wh = wh.astype(ml_dtypes.bfloat16)

    cons = np.zeros((128, 8), dtype=np.float32)
    cons[:NQ, 0] = c
    cons[:NQ, 1] = -c * np.float32(0.01)
    cons[:NQ, 2] = np.float32(0.01)
    cons[:, 3] = np.float32(-np.pi)
    cons[:, 4] = np.float32(np.pi / 2)
    qp = (np.arange(NQ, dtype=np.float64) / (2 * np.pi)).astype(np.float32).reshape(1, NQ)
    idm = np.eye(128, dtype=np.float32)
    shared = dict(wstat_f=wf, wstat_h=wh, cons=cons, qpi=qp, ident=idm)
    maps = []
    for ci in range(N_CORES):
        xs = x[ci * BL:(ci + 1) * BL, :]            # [512, 100]
        xtc = np.ascontiguousarray(xs.T.astype(np.float32))  # [100, 512]
        maps.append(dict(shared, xt=xtc))
    return maps


def _run(x, W, trace=False, trace_kwargs=None):
    global _PROG
    if _PROG is None:
        _PROG = _build_program()
    maps = _host_inputs(x, W)
    res = run_bass_kernel_spmd(_PROG, maps, list(range(N_CORES)),
                               trace=trace, **(trace_kwargs or {}))
    out = np.zeros((B_FULL, D_OUT), dtype=np.float32)
    for ci in range(N_CORES):
        out[ci * BL:(ci + 1) * BL, :NQ] = res.results[ci]["out_qb"].T
    return out, res


def kernel(x, inner_coefficients, theta_matrix, dimension):
    x = np.asarray(x, dtype=np.float32)
    W = np.asarray(inner_coefficients, dtype=np.float32)
    out, _ = _run(x, W, trace=False)
    return out


# revision 24
# speedup vs baseline: 1.0535x; 1.0002x over previous
"""Trainium2 Bass kernel for nn_NoncommutativeKATRepresentation.

Math: out[b,q] = T_q(xm_b) * c_q * s[b,q] / 100 for q=0..99, where
  s[b,q]   = sum_p sin(pi*(q+1)*x[b,p]) * W[q,p]
  c_q      = coeff_q * exp(-0.03*(q+1)^2)            (f32; == 0 for q >= 58)
  xm_b     = mean_q tanh(c_q * s[b,q])
  T_q      = Chebyshev polynomial = cos(q * arccos(xm))
The theta/star-product correction in the reference is exactly zero.
Columns q >= 58 are exactly zero in f32 (exp underflow), so the device
computes only 58 harmonics.

Sharding: pure data parallel over the batch dim across 8 cores (512 rows
each). x is passed pre-transposed per core as [100, 512] (p on partitions,
batch on the free dim). The contraction over p runs on the tensor engine
as 58 accumulating matmuls with one-hot-weighted stationaries.

HW notes that shaped this implementation:
 - ACT `Sin` has NO range reduction: args must be within [-pi, pi].
   Per k we compute u = (k*x) mod 2 in one fused DVE tensor_scalar op and
   evaluate sin(pi*u - pi) = -sin(pi*k*x); the sign is folded into
   negated W stationaries.
 - fp32 matmul costs 4 cycles/row, bf16 1 cycle/row. k <= 15 uses fp32;
   k > 15 uses bf16 (those columns are <= 3e-5 of the output scale, and
   their x_mean contribution is < 1e-7).
 - T_q via closed form cos(q*arccos(xm)) (|xm| <= 0.06): q*theta built by
   a rank-1 matmul, range-reduced with the triangle-wave identity
   cos(pi*p) = cos(pi*|((p+1) mod 2) - 1|).
"""

import numpy as np
import ml_dtypes

import concourse.bacc as bacc
import concourse.tile as tile
import concourse.mybir as mybir
from concourse.bass_utils import run_bass_kernel_spmd

F32 = mybir.dt.float32
BF16 = mybir.dt.bfloat16
AF = mybir.ActivationFunctionType
OP = mybir.AluOpType

N_CORES = 8
B_FULL = 4096
IN = 100
D_OUT = 100
BL = B_FULL // N_CORES          # 512 batch rows per core
KMAX = 58                       # harmonics with nonzero f32 exp term
K_EXACT = 15                    # k <= K_EXACT contracted in fp32
NQ = KMAX                       # output columns computed on device
PI = float(np.pi)

_PROG = None  # cached compiled Bass program


def _coeffs():
    """c_q = coeff_q * exp_term_q computed in f32, mimicking the reference."""
    q = np.arange(KMAX, dtype=np.float32)
    k = (q + np.float32(1.0)).astype(np.float32)
    with np.errstate(divide="ignore", invalid="ignore"):
        s = np.float32(1.0) + np.float32(0.2) * np.log(
            q / np.float32(15.0), dtype=np.float32
        ) * (
            np.float32(1.0)
            - np.exp(np.float32(-0.03) * (q - np.float32(15.0)), dtype=np.float32)
        )
    coeff = np.where(q < 15, np.float32(1.0) / k, np.float32(1.0) / (k * s))
    coeff = coeff.astype(np.float32)
    exp_term = np.exp(np.float32(-0.03) * k * k, dtype=np.float32)
    return (coeff * exp_term).astype(np.float32)


def _build_program():
    nc = bacc.Bacc("TRN2", target_bir_lowering=False, debug=False,
                   num_devices=N_CORES)

    xt = nc.dram_tensor("xt", [IN, BL], F32, kind="ExternalInput")
    wstat_f = nc.dram_tensor("wstat_f", [IN, K_EXACT * NQ], F32,
                             kind="ExternalInput")
    wstat_h = nc.dram_tensor("wstat_h", [IN, (KMAX - K_EXACT) * NQ], BF16,
                             kind="ExternalInput")
    cons = nc.dram_tensor("cons", [128, 8], F32, kind="ExternalInput")
    qpi = nc.dram_tensor("qpi", [1, NQ], F32, kind="ExternalInput")
    ident = nc.dram_tensor("ident", [128, 128], F32, kind="ExternalInput")
    out_qb = nc.dram_tensor("out_qb", [NQ, BL], F32, kind="ExternalOutput")

    with tile.TileContext(nc) as tc:
        with (
            tc.tile_pool(name="const", bufs=1) as cpool,
            tc.tile_pool(name="upool", bufs=5) as upool,
            tc.tile_pool(name="sinf", bufs=3) as sinf_pool,
            tc.tile_pool(name="sinh", bufs=3) as sinh_pool,
            tc.tile_pool(name="epi", bufs=1) as epi,
            tc.tile_pool(name="small", bufs=1) as small,
            tc.tile_pool(name="ps", bufs=1, space="PSUM") as ps,
        ):
            t_xt = cpool.tile([IN, BL], F32)
            t_wf = cpool.tile([IN, K_EXACT * NQ], F32)
            t_wh = cpool.tile([IN, (KMAX - K_EXACT) * NQ], BF16)
            t_cons = cpool.tile([128, 8], F32)
            t_qpi = cpool.tile([1, NQ], F32)
            t_id = cpool.tile([128, 128], F32)
            nc.sync.dma_start(t_xt[:], xt[:])
            nc.sync.dma_start(t_wf[:], wstat_f[:])
            nc.sync.dma_start(t_wh[:], wstat_h[:])
            nc.sync.dma_start(t_cons[:], cons[:])
            nc.sync.dma_start(t_qpi[:], qpi[:])
            nc.sync.dma_start(t_id[:], ident[:])

            # ---- main loop: s[q, b] accumulated in PSUM ----
            # v = (k/2)*x - rint((k/2)*x)  in [-0.5, 0.5];
            # sin(pi*k*x) = sin(2*pi*v).  rint via the 1.5*2^23 magic pair.
            # The rint / subtract / Sin ops are fused across groups of up
            # to 4 harmonics (identical scale 2*pi) to amortize per-op
            # fixed overheads; subtracts alternate between GpSimd and DVE.
            MAGIC = 12582912.0
            S_ps = ps.tile([NQ, BL], F32)

            def mm(i, sbuf_slice):
                k = i + 1
                if k <= K_EXACT:
                    stat = t_wf[:, i * NQ:(i + 1) * NQ]
                else:
                    stat = t_wh[:, (i - K_EXACT) * NQ:(i - K_EXACT + 1) * NQ]
                nc.tensor.matmul(S_ps[:], stat, sbuf_slice,
                                 start=(i == 0), stop=(i == KMAX - 1))

            # k = 1 directly: pi*x in [0, pi)
            sk1 = sinf_pool.tile([IN, BL], F32, tag="sf1")
            nc.scalar.activation(sk1[:], t_xt[:], AF.Sin, scale=PI)
            mm(0, sk1[:])

            groups = []
            ks = list(range(2, K_EXACT + 1))
            groups += [ks[j:j + 4] for j in range(0, len(ks), 4)]
            ks = list(range(K_EXACT + 1, KMAX + 1))
            groups += [ks[j:j + 4] for j in range(0, len(ks), 4)]
            for gi, grp in enumerate(groups):
                g = len(grp)
                hb = upool.tile([IN, g * BL], F32, tag="h")
                for j, k in enumerate(grp):
                    nc.vector.tensor_scalar(hb[:, j * BL:(j + 1) * BL],
                                            t_xt[:], 0.5 * k, None, OP.mult)
                rb = upool.tile([IN, g * BL], F32, tag="rh")
                nc.vector.tensor_scalar(rb[:], hb[:], MAGIC, MAGIC,
                                        OP.add, OP.subtract)
                vb = upool.tile([IN, g * BL], F32, tag="v")
                if gi not in (0, 5, 9, 13):
                    nc.gpsimd.tensor_sub(vb[:], hb[:], rb[:])
                else:
                    nc.vector.tensor_sub(vb[:], hb[:], rb[:])
                if grp[0] <= K_EXACT:
                    sb = sinf_pool.tile([IN, g * BL], F32, tag="sf")
                else:
                    sb = sinh_pool.tile([IN, g * BL], BF16, tag="sh")
                nc.scalar.activation(sb[:], vb[:], AF.Sin, scale=2.0 * PI)
                for j, k in enumerate(grp):
                    mm(k - 1, sb[:, j * BL:(j + 1) * BL])

            # ---- epilogue ----
            # xn = tanh(c_q * s); Sc = (c_q/100) * s
            t_xn = epi.tile([NQ, BL], F32)
            nc.scalar.activation(t_xn[:], S_ps[:], AF.Tanh,
                                 scale=t_cons[:NQ, 0:1])
            t_sc = epi.tile([NQ, BL], F32)
            nc.scalar.activation(t_sc[:], S_ps[:], AF.Copy,
                                 scale=t_cons[:NQ, 1:2])

            # x_mean directly in [128, 4] layout: one matmul per 128-batch
            # chunk with xn-chunk as the stationary operand.
            xm_ps = ps.tile([128, 4], F32)
            for j in range(4):
                nc.tensor.matmul(xm_ps[:, j:j + 1],
                                 t_xn[:, j * 128:(j + 1) * 128],
                                 t_cons[:NQ, 2:3], start=True, stop=True)

            t_m = small.tile([128, 4], F32)
            nc.vector.tensor_copy(t_m[:], xm_ps[:])
            t_mc = small.tile([128, 4], F32)
            nc.vector.tensor_scalar(t_mc[:], t_m[:], 0.5, -0.5, OP.min, OP.max)
            # theta0 = pi/2 - m - m^3/6
            t_m2 = small.tile([128, 4], F32)
            nc.vector.tensor_mul(t_m2[:], t_mc[:], t_mc[:])
            t_m3 = small.tile([128, 4], F32)
            nc.vector.tensor_mul(t_m3[:], t_m2[:], t_mc[:])
            t_a = small.tile([128, 4], F32)
            nc.vector.tensor_scalar(t_a[:], t_m3[:], -1.0 / 6.0, PI / 2.0,
                                    OP.mult, OP.add)
            t_th = small.tile([128, 4], F32)
            nc.vector.tensor_sub(t_th[:], t_a[:], t_mc[:])
            # one Newton step: theta += (cos th - m) / sin th
            t_sth = small.tile([128, 4], F32)
            nc.scalar.activation(t_sth[:], t_th[:], AF.Sin)
            t_cth = small.tile([128, 4], F32)
            nc.scalar.activation(t_cth[:], t_th[:], AF.Sin,
                                 bias=t_cons[:, 4:5])
            t_r = small.tile([128, 4], F32)
            nc.vector.reciprocal(t_r[:], t_sth[:])
            t_d = small.tile([128, 4], F32)
            nc.vector.tensor_sub(t_d[:], t_cth[:], t_mc[:])
            t_e = small.tile([128, 4], F32)
            nc.vector.tensor_mul(t_e[:], t_d[:], t_r[:])
            t_th2 = small.tile([128, 4], F32)
            nc.vector.tensor_add(t_th2[:], t_th[:], t_e[:])

            # theta as 4 rows of [1, 128] via per-column PE transposes
            t_throws = []
            for j in range(4):
                tp_ps = ps.tile([1, 128], F32, tag="tp")
                nc.tensor.transpose(tp_ps[:], t_th2[:, j:j + 1], t_id[:])
                tr = small.tile([1, 128], F32, tag=f"thr{j}")
                nc.vector.tensor_copy(tr[:], tp_ps[:])
                t_throws.append(tr)

            # phase[q, b] = (q/pi) * theta_b ; 4 rank-1 matmuls
            ph_ps = ps.tile([NQ, BL], F32)
            for j in range(4):
                nc.tensor.matmul(ph_ps[:, j * 128:(j + 1) * 128],
                                 t_qpi[:], t_throws[j][0:1, :],
                                 start=True, stop=True)
            # ph = q*theta/(2*pi).  T_q = cos(2*pi*ph) = -sin(2*pi*u) with
            # u = (ph - 1/4) - rint(ph - 1/4); the -1 is folded into cons[:,1].
            t_e = epi.tile([NQ, BL], F32)
            nc.vector.tensor_scalar(t_e[:], ph_ps[:], -0.25, None, OP.add)
            t_re = epi.tile([NQ, BL], F32)
            nc.vector.tensor_scalar(t_re[:], t_e[:], MAGIC, MAGIC,
                                    OP.add, OP.subtract)
            t_u = epi.tile([NQ, BL], F32)
            nc.vector.tensor_sub(t_u[:], t_e[:], t_re[:])
            t_outer = epi.tile([NQ, BL], F32)
            nc.scalar.activation(t_outer[:], t_u[:], AF.Sin, scale=2.0 * PI)
            t_res = epi.tile([NQ, BL], F32)
            nc.vector.tensor_mul(t_res[:], t_outer[:], t_sc[:])
            nc.sync.dma_start(out_qb[:], t_res[:])

    nc.compile()
    return nc


def _prepare_static_inputs():
    """Inputs that do not depend on x (built once)."""
    return None


def _host_inputs(x, W):
    c = _coeffs()
    negW = (W[:KMAX, :IN]).astype(np.float32)       # [58, 100]

    wf = np.zeros((IN, K_EXACT * NQ), dtype=np.float32)
    for i in range(K_EXACT):
        wf[:, i * NQ + i] = negW[i, :]
    wh = np.zeros((IN, (KMAX - K_EXACT) * NQ), dtype=np.float32)
    for i in range(K_EXACT, KMAX):
        j = i - K_EXACT
        wh[:, j * NQ + i] = negW[i, :]
    wh = wh.astype(ml_dtypes.bfloat16)

    cons = np.zeros((128, 8), dtype=np.float32)
    cons[:NQ, 0] = c
    cons[:NQ, 1] = -c * np.float32(0.01)
    cons[:NQ, 2] = np.float32(0.01)
    cons[:, 3] = np.float32(-np.pi)
    cons[:, 4] = np.float32(np.pi / 2)
    qp = (np.arange(NQ, dtype=np.float64) / (2 * np.pi)).astype(np.float32).reshape(1, NQ)
    idm = np.eye(128, dtype=np.float32)
    shared = dict(wstat_f=wf, wstat_h=wh, cons=cons, qpi=qp, ident=idm)
    maps = []
    for ci in range(N_CORES):
        xs = x[ci * BL:(ci + 1) * BL, :]            # [512, 100]
        xtc = np.ascontiguousarray(xs.T.astype(np.float32))  # [100, 512]
        maps.append(dict(shared, xt=xtc))
    return maps


def _run(x, W, trace=False, trace_kwargs=None):
    global _PROG
    if _PROG is None:
        _PROG = _build_program()
    maps = _host_inputs(x, W)
    res = run_bass_kernel_spmd(_PROG, maps, list(range(N_CORES)),
                               trace=trace, **(trace_kwargs or {}))
    out = np.zeros((B_FULL, D_OUT), dtype=np.float32)
    for ci in range(N_CORES):
        out[ci * BL:(ci + 1) * BL, :NQ] = res.results[ci]["out_qb"].T
    return out, res


def kernel(x, inner_coefficients, theta_matrix, dimension):
    x = np.asarray(x, dtype=np.float32)
    W = np.asarray(inner_coefficients, dtype=np.float32)
    out, _ = _run(x, W, trace=False)
    return out


# revision 29
# speedup vs baseline: 1.1130x; 1.0565x over previous
"""Trainium2 Bass kernel for nn_NoncommutativeKATRepresentation.

Math: out[b,q] = T_q(xm_b) * c_q * s[b,q] / 100 for q=0..99, where
  s[b,q]   = sum_p sin(pi*(q+1)*x[b,p]) * W[q,p]
  c_q      = coeff_q * exp(-0.03*(q+1)^2)            (f32; == 0 for q >= 58)
  xm_b     = mean_q tanh(c_q * s[b,q])
  T_q      = Chebyshev polynomial = cos(q * arccos(xm))
The theta/star-product correction in the reference is exactly zero.
Columns q >= 58 are exactly zero in f32 (exp underflow), so the device
computes only 58 harmonics.

Sharding: pure data parallel over the batch dim across 8 cores (512 rows
each). x is passed pre-transposed per core as [100, 512] (p on partitions,
batch on the free dim). The contraction over p runs on the tensor engine
as 58 accumulating matmuls with one-hot-weighted stationaries.

HW notes that shaped this implementation:
 - ACT `Sin` has NO range reduction: args must be within [-pi, pi].
   Per k we compute u = (k*x) mod 2 in one fused DVE tensor_scalar op and
   evaluate sin(pi*u - pi) = -sin(pi*k*x); the sign is folded into
   negated W stationaries.
 - fp32 matmul costs 4 cycles/row, bf16 1 cycle/row. k <= 15 uses fp32;
   k > 15 uses bf16 (those columns are <= 3e-5 of the output scale, and
   their x_mean contribution is < 1e-7).
 - T_q via closed form cos(q*arccos(xm)) (|xm| <= 0.06): q*theta built by
   a rank-1 matmul, range-reduced with the triangle-wave identity
   cos(pi*p) = cos(pi*|((p+1) mod 2) - 1|).
"""

import numpy as np
import ml_dtypes

import concourse.bacc as bacc
import concourse.tile as tile
import concourse.mybir as mybir
from concourse.bass_utils import run_bass_kernel_spmd

F32 = mybir.dt.float32
BF16 = mybir.dt.bfloat16
AF = mybir.ActivationFunctionType
OP = mybir.AluOpType

N_CORES = 8
B_FULL = 4096
IN = 100
D_OUT = 100
BL = B_FULL // N_CORES          # 512 batch rows per core
KMAX = 58                       # harmonics with nonzero f32 exp term
K_EXACT = 15                    # k <= K_EXACT contracted in fp32
NQ = KMAX                       # output columns computed on device
PI = float(np.pi)

_PROG = None  # cached compiled Bass program


def _coeffs():
    """c_q = coeff_q * exp_term_q computed in f32, mimicking the reference."""
    q = np.arange(KMAX, dtype=np.float32)
    k = (q + np.float32(1.0)).astype(np.float32)
    with np.errstate(divide="ignore", invalid="ignore"):
        s = np.float32(1.0) + np.float32(0.2) * np.log(
            q / np.float32(15.0), dtype=np.float32
        ) * (
            np.float32(1.0)
            - np.exp(np.float32(-0.03) * (q - np.float32(15.0)), dtype=np.float32)
        )
    coeff = np.where(q < 15, np.float32(1.0) / k, np.float32(1.0) / (k * s))
    coeff = coeff.astype(np.float32)
    exp_term = np.exp(np.float32(-0.03) * k * k, dtype=np.float32)
    return (coeff * exp_term).astype(np.float32)


def _build_program():
    nc = bacc.Bacc("TRN2", target_bir_lowering=False, debug=False,
                   num_devices=N_CORES)

    xt = nc.dram_tensor("xt", [IN, BL], F32, kind="ExternalInput")
    wstat_f = nc.dram_tensor("wstat_f", [IN, K_EXACT * 32], F32,
                             kind="ExternalInput")
    wstat_h = nc.dram_tensor("wstat_h", [IN, 17 * 32 + 26 * 26], BF16,
                             kind="ExternalInput")
    cons = nc.dram_tensor("cons", [128, 8], F32, kind="ExternalInput")
    qpi = nc.dram_tensor("qpi", [1, NQ], F32, kind="ExternalInput")
    ident = nc.dram_tensor("ident", [128, 128], F32, kind="ExternalInput")
    out_qb = nc.dram_tensor("out_qb", [NQ, BL], F32, kind="ExternalOutput")

    with tile.TileContext(nc) as tc:
        with (
            tc.tile_pool(name="const", bufs=1) as cpool,
            tc.tile_pool(name="upool", bufs=5) as upool,
            tc.tile_pool(name="sinf", bufs=3) as sinf_pool,
            tc.tile_pool(name="sinh", bufs=3) as sinh_pool,
            tc.tile_pool(name="epi", bufs=1) as epi,
            tc.tile_pool(name="small", bufs=1) as small,
            tc.tile_pool(name="ps", bufs=1, space="PSUM") as ps,
        ):
            t_xt = cpool.tile([IN, BL], F32)
            t_wf = cpool.tile([IN, K_EXACT * 32], F32)
            t_wh = cpool.tile([IN, 17 * 32 + 26 * 26], BF16)
            t_cons = cpool.tile([128, 8], F32)
            t_qpi = cpool.tile([1, NQ], F32)
            t_id = cpool.tile([128, 128], F32)
            nc.sync.dma_start(t_xt[:], xt[:])
            nc.sync.dma_start(t_wf[:], wstat_f[:])
            nc.sync.dma_start(t_wh[:], wstat_h[:])
            nc.sync.dma_start(t_cons[:], cons[:])
            nc.sync.dma_start(t_qpi[:], qpi[:])
            nc.sync.dma_start(t_id[:], ident[:])

            # ---- main loop: s[q, b] accumulated in PSUM ----
            # v = (k/2)*x - rint((k/2)*x)  in [-0.5, 0.5];
            # sin(pi*k*x) = sin(2*pi*v).  rint via the 1.5*2^23 magic pair.
            # The rint / subtract / Sin ops are fused across groups of up
            # to 4 harmonics (identical scale 2*pi) to amortize per-op
            # fixed overheads; subtracts alternate between GpSimd and DVE.
            MAGIC = 12582912.0
            # split accumulator: S_lo (k=1..32) finishes early so the
            # x_mean/theta/outer chain overlaps the k=33..58 matmuls.
            S_lo = ps.tile([32, BL], F32)
            S_hi = ps.tile([KMAX - 32, BL], F32)

            def mm(i, sbuf_slice):
                k = i + 1
                out = S_lo if k <= 32 else S_hi
                # stationary layout: f32 block [100, 15*32]; bf16 block:
                # k=16..32 at [100, 17*32], then k=33..58 at [100, 26*26]
                if k <= K_EXACT:
                    stat = t_wf[:, i * 32:(i + 1) * 32]
                elif k <= 32:
                    j = i - K_EXACT
                    stat = t_wh[:, j * 32:(j + 1) * 32]
                else:
                    j = i - 32
                    base = 17 * 32
                    stat = t_wh[:, base + j * 26:base + (j + 1) * 26]
                nc.tensor.matmul(out[:], stat, sbuf_slice,
                                 start=(k == 1 or k == 33),
                                 stop=(k == 32 or k == KMAX))

            # k = 1 directly: pi*x in [0, pi)
            sk1 = sinf_pool.tile([IN, BL], F32, tag="sf1")
            nc.scalar.activation(sk1[:], t_xt[:], AF.Sin, scale=PI)
            mm(0, sk1[:])

            groups = []
            ks = list(range(2, K_EXACT + 1))
            groups += [ks[j:j + 4] for j in range(0, len(ks), 4)]
            ks = list(range(K_EXACT + 1, KMAX + 1))
            groups += [ks[j:j + 4] for j in range(0, len(ks), 4)]
            # process slow (GpSimd-subtract) groups first so the loop tail
            # ends on fast DVE groups; PSUM accumulation is order-free.
            dve_sub = (0, 5, 9, 13, 14)
            for gi, grp in enumerate(groups):
                g = len(grp)
                hb = upool.tile([IN, g * BL], F32, tag="h")
                for j, k in enumerate(grp):
                    nc.vector.tensor_scalar(hb[:, j * BL:(j + 1) * BL],
                                            t_xt[:], 0.5 * k, None, OP.mult)
                rb = upool.tile([IN, g * BL], F32, tag="rh")
                nc.vector.tensor_scalar(rb[:], hb[:], MAGIC, MAGIC,
                                        OP.add, OP.subtract)
                vb = upool.tile([IN, g * BL], F32, tag="v")
                if gi not in dve_sub:
                    nc.gpsimd.tensor_sub(vb[:], hb[:], rb[:])
                else:
                    nc.vector.tensor_sub(vb[:], hb[:], rb[:])
                if grp[0] <= K_EXACT:
                    sb = sinf_pool.tile([IN, g * BL], F32, tag="sf")
                else:
                    sb = sinh_pool.tile([IN, g * BL], BF16, tag="sh")
                nc.scalar.activation(sb[:], vb[:], AF.Sin, scale=2.0 * PI)
                for j, k in enumerate(grp):
                    mm(k - 1, sb[:, j * BL:(j + 1) * BL])

            # ---- epilogue ----
            # xn = tanh(c_q * s); Sc = (c_q/100) * s
            t_xn = epi.tile([32, BL], F32)
            nc.scalar.activation(t_xn[:], S_lo[:], AF.Tanh,
                                 scale=t_cons[:32, 0:1])
            t_sc = epi.tile([NQ, BL], F32)
            nc.scalar.activation(t_sc[0:32, :], S_lo[:], AF.Copy,
                                 scale=t_cons[0:32, 1:2])
            nc.scalar.activation(t_sc[32:NQ, :], S_hi[:], AF.Copy,
                                 scale=t_cons[32:NQ, 1:2])

            # x_mean directly in [128, 4] layout: one matmul per 128-batch
            # chunk with xn-chunk as the stationary operand.
            xm_ps = ps.tile([128, 4], F32)
            for j in range(4):
                nc.tensor.matmul(xm_ps[:, j:j + 1],
                                 t_xn[:, j * 128:(j + 1) * 128],
                                 t_cons[:32, 2:3], start=True, stop=True)

            t_m = small.tile([128, 4], F32)
            nc.vector.tensor_copy(t_m[:], xm_ps[:])
            t_mc = small.tile([128, 4], F32)
            nc.vector.tensor_scalar(t_mc[:], t_m[:], 0.5, -0.5, OP.min, OP.max)
            # theta0 = pi/2 - m - m^3/6
            t_m2 = small.tile([128, 4], F32)
            nc.vector.tensor_mul(t_m2[:], t_mc[:], t_mc[:])
            t_m3 = small.tile([128, 4], F32)
            nc.vector.tensor_mul(t_m3[:], t_m2[:], t_mc[:])
            t_a = small.tile([128, 4], F32)
            nc.vector.tensor_scalar(t_a[:], t_m3[:], -1.0 / 6.0, PI / 2.0,
                                    OP.mult, OP.add)
            t_th = small.tile([128, 4], F32)
            nc.vector.tensor_sub(t_th[:], t_a[:], t_mc[:])
            # one Newton step: theta += (cos th - m) / sin th
            t_sth = small.tile([128, 4], F32)
            nc.scalar.activation(t_sth[:], t_th[:], AF.Sin)
            t_cth = small.tile([128, 4], F32)
            nc.scalar.activation(t_cth[:], t_th[:], AF.Sin,
                                 bias=t_cons[:, 4:5])
            t_r = small.tile([128, 4], F32)
            nc.vector.reciprocal(t_r[:], t_sth[:])
            t_d = small.tile([128, 4], F32)
            nc.vector.tensor_sub(t_d[:], t_cth[:], t_mc[:])
            t_e = small.tile([128, 4], F32)
            nc.vector.tensor_mul(t_e[:], t_d[:], t_r[:])
            t_th2 = small.tile([128, 4], F32)
            nc.vector.tensor_add(t_th2[:], t_th[:], t_e[:])

            # theta as 4 rows of [1, 128] via per-column PE transposes
            t_throws = []
            for j in range(4):
                tp_ps = ps.tile([1, 128], F32, tag="tp")
                nc.tensor.transpose(tp_ps[:], t_th2[:, j:j + 1], t_id[:])
                tr = small.tile([1, 128], F32, tag=f"thr{j}")
                nc.vector.tensor_copy(tr[:], tp_ps[:])
                t_throws.append(tr)

            # phase[q, b] = (q/pi) * theta_b ; 4 rank-1 matmuls
            ph_ps = ps.tile([NQ, BL], F32)
            for j in range(4):
                nc.tensor.matmul(ph_ps[:, j * 128:(j + 1) * 128],
                                 t_qpi[:], t_throws[j][0:1, :],
                                 start=True, stop=True)
            # ph = q*theta/(2*pi).  T_q = cos(2*pi*ph) = -sin(2*pi*u) with
            # u = (ph - 1/4) - rint(ph - 1/4); the -1 is folded into cons[:,1].
            t_e = epi.tile([NQ, BL], F32)
            nc.vector.tensor_scalar(t_e[:], ph_ps[:], -0.25, None, OP.add)
            t_re = epi.tile([NQ, BL], F32)
            nc.vector.tensor_scalar(t_re[:], t_e[:], MAGIC, MAGIC,
                                    OP.add, OP.subtract)
            t_u = epi.tile([NQ, BL], F32)
            nc.vector.tensor_sub(t_u[:], t_e[:], t_re[:])
            t_outer = epi.tile([NQ, BL], F32)
            nc.scalar.activation(t_outer[:], t_u[:], AF.Sin, scale=2.0 * PI)
            t_res = epi.tile([NQ, BL], F32)
            nc.vector.tensor_mul(t_res[:], t_outer[:], t_sc[:])
            nc.sync.dma_start(out_qb[:], t_res[:])

    nc.compile()
    return nc


def _prepare_static_inputs():
    """Inputs that do not depend on x (built once)."""
    return None


def _host_inputs(x, W):
    c = _coeffs()
    negW = (W[:KMAX, :IN]).astype(np.float32)       # [58, 100]

    # f32 block: k=1..15, one-hot in 32 cols (out partitions 0..31)
    wf = np.zeros((IN, K_EXACT * 32), dtype=np.float32)
    for i in range(K_EXACT):
        wf[:, i * 32 + i] = negW[i, :]
    # bf16 block: k=16..32 one-hot in 32 cols, then k=33..58 in 26 cols
    wh = np.zeros((IN, 17 * 32 + 26 * 26), dtype=np.float32)
    for i in range(K_EXACT, 32):
        j = i - K_EXACT
        wh[:, j * 32 + i] = negW[i, :]
    for i in range(32, KMAX):
        j = i - 32
        wh[:, 17 * 32 + j * 26 + j] = negW[i, :]
    wh = wh.astype(ml_dtypes.bfloat16)

    cons = np.zeros((128, 8), dtype=np.float32)
    cons[:NQ, 0] = c
    cons[:NQ, 1] = -c * np.float32(0.01)
    cons[:NQ, 2] = np.float32(0.01)
    cons[:, 3] = np.float32(-np.pi)
    cons[:, 4] = np.float32(np.pi / 2)
    qp = (np.arange(NQ, dtype=np.float64) / (2 * np.pi)).astype(np.float32).reshape(1, NQ)
    idm = np.eye(128, dtype=np.float32)
    shared = dict(wstat_f=wf, wstat_h=wh, cons=cons, qpi=qp, ident=idm)
    maps = []
    for ci in range(N_CORES):
        xs = x[ci * BL:(ci + 1) * BL, :]            # [512, 100]
        xtc = np.ascontiguousarray(xs.T.astype(np.float32))  # [100, 512]
        maps.append(dict(shared, xt=xtc))
    return maps


def _run(x, W, trace=False, trace_kwargs=None):
    global _PROG
    if _PROG is None:
        _PROG = _build_program()
    maps = _host_inputs(x, W)
    res = run_bass_kernel_spmd(_PROG, maps, list(range(N_CORES)),
                               trace=trace, **(trace_kwargs or {}))
    out = np.zeros((B_FULL, D_OUT), dtype=np.float32)
    for ci in range(N_CORES):
        out[ci * BL:(ci + 1) * BL, :NQ] = res.results[ci]["out_qb"].T
    return out, res


def kernel(x, inner_coefficients, theta_matrix, dimension):
    x = np.asarray(x, dtype=np.float32)
    W = np.asarray(inner_coefficients, dtype=np.float32)
    out, _ = _run(x, W, trace=False)
    return out


# revision 32
# speedup vs baseline: 1.1838x; 1.0637x over previous
"""Trainium2 Bass kernel for nn_NoncommutativeKATRepresentation.

Math: out[b,q] = T_q(xm_b) * c_q * s[b,q] / 100 for q=0..99, where
  s[b,q]   = sum_p sin(pi*(q+1)*x[b,p]) * W[q,p]
  c_q      = coeff_q * exp(-0.03*(q+1)^2)            (f32; == 0 for q >= 58)
  xm_b     = mean_q tanh(c_q * s[b,q])
  T_q      = Chebyshev polynomial = cos(q * arccos(xm))
The theta/star-product correction in the reference is exactly zero.
Columns q >= 58 are exactly zero in f32 (exp underflow), so the device
computes only 58 harmonics.

Sharding: pure data parallel over the batch dim across 8 cores (512 rows
each). x is passed pre-transposed per core as [100, 512] (p on partitions,
batch on the free dim). The contraction over p runs on the tensor engine
as 58 accumulating matmuls with one-hot-weighted stationaries.

HW notes that shaped this implementation:
 - ACT `Sin` has NO range reduction: args must be within [-pi, pi].
   Per k we compute u = (k*x) mod 2 in one fused DVE tensor_scalar op and
   evaluate sin(pi*u - pi) = -sin(pi*k*x); the sign is folded into
   negated W stationaries.
 - fp32 matmul costs 4 cycles/row, bf16 1 cycle/row. k <= 15 uses fp32;
   k > 15 uses bf16 (those columns are <= 3e-5 of the output scale, and
   their x_mean contribution is < 1e-7).
 - T_q via closed form cos(q*arccos(xm)) (|xm| <= 0.06): q*theta built by
   a rank-1 matmul, range-reduced with the triangle-wave identity
   cos(pi*p) = cos(pi*|((p+1) mod 2) - 1|).
"""

import numpy as np
import ml_dtypes

import concourse.bacc as bacc
import concourse.tile as tile
import concourse.mybir as mybir
from concourse.bass_utils import run_bass_kernel_spmd

F32 = mybir.dt.float32
BF16 = mybir.dt.bfloat16
AF = mybir.ActivationFunctionType
OP = mybir.AluOpType

N_CORES = 8
B_FULL = 4096
IN = 100
D_OUT = 100
BL = B_FULL // N_CORES          # 512 batch rows per core
KMAX = 58                       # harmonics with nonzero f32 exp term
K_EXACT = 15                    # k <= K_EXACT contracted in fp32
NQ = KMAX                       # output columns computed on device
PI = float(np.pi)

_PROG = None  # cached compiled Bass program


def _coeffs():
    """c_q = coeff_q * exp_term_q computed in f32, mimicking the reference."""
    q = np.arange(KMAX, dtype=np.float32)
    k = (q + np.float32(1.0)).astype(np.float32)
    with np.errstate(divide="ignore", invalid="ignore"):
        s = np.float32(1.0) + np.float32(0.2) * np.log(
            q / np.float32(15.0), dtype=np.float32
        ) * (
            np.float32(1.0)
            - np.exp(np.float32(-0.03) * (q - np.float32(15.0)), dtype=np.float32)
        )
    coeff = np.where(q < 15, np.float32(1.0) / k, np.float32(1.0) / (k * s))
    coeff = coeff.astype(np.float32)
    exp_term = np.exp(np.float32(-0.03) * k * k, dtype=np.float32)
    return (coeff * exp_term).astype(np.float32)


def _build_program():
    nc = bacc.Bacc("TRN2", target_bir_lowering=False, debug=False,
                   num_devices=N_CORES)

    xt = nc.dram_tensor("xt", [IN, BL], F32, kind="ExternalInput")
    wstat_f = nc.dram_tensor("wstat_f", [IN, K_EXACT * 32], F32,
                             kind="ExternalInput")
    wstat_h = nc.dram_tensor("wstat_h", [IN, 17 * 32 + 26 * 26], BF16,
                             kind="ExternalInput")
    cons = nc.dram_tensor("cons", [128, 8], F32, kind="ExternalInput")
    qpi = nc.dram_tensor("qpi", [1, NQ], F32, kind="ExternalInput")
    ident = nc.dram_tensor("ident", [128, 128], F32, kind="ExternalInput")
    out_qb = nc.dram_tensor("out_qb", [NQ, BL], F32, kind="ExternalOutput")

    with tile.TileContext(nc) as tc:
        with (
            tc.tile_pool(name="const", bufs=1) as cpool,
            tc.tile_pool(name="upool", bufs=8) as upool,
            tc.tile_pool(name="sinf", bufs=4) as sinf_pool,
            tc.tile_pool(name="sinh", bufs=4) as sinh_pool,
            tc.tile_pool(name="epi", bufs=1) as epi,
            tc.tile_pool(name="small", bufs=1) as small,
            tc.tile_pool(name="ps", bufs=1, space="PSUM") as ps,
        ):
            t_xt = cpool.tile([IN, BL], F32)
            t_wf = cpool.tile([IN, K_EXACT * 32], F32)
            t_wh = cpool.tile([IN, 17 * 32 + 26 * 26], BF16)
            t_cons = cpool.tile([128, 8], F32)
            t_qpi = cpool.tile([1, NQ], F32)
            t_id = cpool.tile([128, 128], F32)
            nc.sync.dma_start(t_xt[:], xt[:])
            nc.sync.dma_start(t_wf[:], wstat_f[:])
            nc.sync.dma_start(t_wh[:], wstat_h[:])
            nc.sync.dma_start(t_cons[:], cons[:])
            nc.sync.dma_start(t_qpi[:], qpi[:])
            nc.sync.dma_start(t_id[:], ident[:])

            # ---- main loop: s[q, b] accumulated in PSUM ----
            # v = (k/2)*x - rint((k/2)*x)  in [-0.5, 0.5];
            # sin(pi*k*x) = sin(2*pi*v).  rint via the 1.5*2^23 magic pair.
            # The rint / subtract / Sin ops are fused across groups of up
            # to 4 harmonics (identical scale 2*pi) to amortize per-op
            # fixed overheads; subtracts alternate between GpSimd and DVE.
            MAGIC = 12582912.0
            # split accumulator: S_lo (k=1..32) finishes early so the
            # x_mean/theta/outer chain overlaps the k=33..58 matmuls.
            S_lo = ps.tile([32, BL], F32)
            S_hi = ps.tile([KMAX - 32, BL], F32)

            def mm(i, sbuf_slice):
                k = i + 1
                out = S_lo if k <= 32 else S_hi
                # stationary layout: f32 block [100, 15*32]; bf16 block:
                # k=16..32 at [100, 17*32], then k=33..58 at [100, 26*26]
                if k <= K_EXACT:
                    stat = t_wf[:, i * 32:(i + 1) * 32]
                elif k <= 32:
                    j = i - K_EXACT
                    stat = t_wh[:, j * 32:(j + 1) * 32]
                else:
                    j = i - 32
                    base = 17 * 32
                    stat = t_wh[:, base + j * 26:base + (j + 1) * 26]
                nc.tensor.matmul(out[:], stat, sbuf_slice,
                                 start=(k == 1 or k == 33),
                                 stop=(k == 32 or k == KMAX))

            # k = 1 directly: pi*x in [0, pi)
            sk1 = sinf_pool.tile([IN, BL], F32, tag="sf1")
            nc.scalar.activation(sk1[:], t_xt[:], AF.Sin, scale=PI)
            mm(0, sk1[:])
            # k = 2: sin(2*pi*x - pi) = -sin(2*pi*x); W row 2 pre-negated
            sk2 = sinf_pool.tile([IN, BL], F32, tag="sf2")
            nc.scalar.activation(sk2[:], t_xt[:], AF.Sin,
                                 bias=t_cons[:IN, 3:4], scale=2.0 * PI)
            mm(1, sk2[:])

            groups = []
            ks = list(range(3, K_EXACT + 1))
            groups += [ks[j:j + 2] for j in range(0, len(ks), 2)]
            ks = list(range(K_EXACT + 1, KMAX + 1))
            groups += [ks[j:j + 2] for j in range(0, len(ks), 2)]
            # process slow (GpSimd-subtract) groups first so the loop tail
            # ends on fast DVE groups; PSUM accumulation is order-free.
            dve_sub = tuple(range(0, 30, 3))
            for gi, grp in enumerate(groups):
                g = len(grp)
                hb = upool.tile([IN, g * BL], F32, tag="h")
                for j, k in enumerate(grp):
                    nc.vector.tensor_scalar(hb[:, j * BL:(j + 1) * BL],
                                            t_xt[:], 0.5 * k, None, OP.mult)
                rb = upool.tile([IN, g * BL], F32, tag="rh")
                nc.vector.tensor_scalar(rb[:], hb[:], MAGIC, MAGIC,
                                        OP.add, OP.subtract)
                vb = upool.tile([IN, g * BL], F32, tag="v")
                if gi not in dve_sub:
                    nc.gpsimd.tensor_sub(vb[:], hb[:], rb[:])
                else:
                    nc.vector.tensor_sub(vb[:], hb[:], rb[:])
                if grp[0] <= K_EXACT:
                    sb = sinf_pool.tile([IN, g * BL], F32, tag="sf")
                else:
                    sb = sinh_pool.tile([IN, g * BL], BF16, tag="sh")
                nc.scalar.activation(sb[:], vb[:], AF.Sin, scale=2.0 * PI)
                for j, k in enumerate(grp):
                    mm(k - 1, sb[:, j * BL:(j + 1) * BL])

            # ---- epilogue ----
            # xn = tanh(c_q * s); Sc = (c_q/100) * s
            t_xn = epi.tile([32, BL], F32)
            nc.scalar.activation(t_xn[:], S_lo[:], AF.Tanh,
                                 scale=t_cons[:32, 0:1])
            t_sc = epi.tile([NQ, BL], F32)
            nc.scalar.activation(t_sc[0:32, :], S_lo[:], AF.Copy,
                                 scale=t_cons[0:32, 1:2])
            nc.scalar.activation(t_sc[32:NQ, :], S_hi[:], AF.Copy,
                                 scale=t_cons[32:NQ, 1:2])

            # x_mean directly in [128, 4] layout: one matmul per 128-batch
            # chunk with xn-chunk as the stationary operand.
            xm_ps = ps.tile([128, 4], F32)
            for j in range(4):
                nc.tensor.matmul(xm_ps[:, j:j + 1],
                                 t_xn[:, j * 128:(j + 1) * 128],
                                 t_cons[:32, 2:3], start=True, stop=True)

            t_m = small.tile([128, 4], F32)
            nc.vector.tensor_copy(t_m[:], xm_ps[:])
            t_mc = small.tile([128, 4], F32)
            nc.vector.tensor_scalar(t_mc[:], t_m[:], 0.5, -0.5, OP.min, OP.max)
            # theta0 = pi/2 - m - m^3/6
            t_m2 = small.tile([128, 4], F32)
            nc.vector.tensor_mul(t_m2[:], t_mc[:], t_mc[:])
            t_m3 = small.tile([128, 4], F32)
            nc.vector.tensor_mul(t_m3[:], t_m2[:], t_mc[:])
            t_a = small.tile([128, 4], F32)
            nc.vector.tensor_scalar(t_a[:], t_m3[:], -1.0 / 6.0, PI / 2.0,
                                    OP.mult, OP.add)
            t_th = small.tile([128, 4], F32)
            nc.vector.tensor_sub(t_th[:], t_a[:], t_mc[:])
            # one Newton step: theta += (cos th - m) / sin th
            t_sth = small.tile([128, 4], F32)
            nc.scalar.activation(t_sth[:], t_th[:], AF.Sin)
            t_cth = small.tile([128, 4], F32)
            nc.scalar.activation(t_cth[:], t_th[:], AF.Sin,
                                 bias=t_cons[:, 4:5])
            t_r = small.tile([128, 4], F32)
            nc.vector.reciprocal(t_r[:], t_sth[:])
            t_d = small.tile([128, 4], F32)
            nc.vector.tensor_sub(t_d[:], t_cth[:], t_mc[:])
            t_e = small.tile([128, 4], F32)
            nc.vector.tensor_mul(t_e[:], t_d[:], t_r[:])
            t_th2 = small.tile([128, 4], F32)
            nc.vector.tensor_add(t_th2[:], t_th[:], t_e[:])

            # theta as 4 rows of [1, 128] via per-column PE transposes
            t_throws = []
            for j in range(4):
                tp_ps = ps.tile([1, 128], F32, tag="tp")
                nc.tensor.transpose(tp_ps[:], t_th2[:, j:j + 1], t_id[:])
                tr = small.tile([1, 128], F32, tag=f"thr{j}")
                nc.vector.tensor_copy(tr[:], tp_ps[:])
                t_throws.append(tr)

            # phase[q, b] = (q/pi) * theta_b ; 4 rank-1 matmuls
            ph_ps = ps.tile([NQ, BL], F32)
            for j in range(4):
                nc.tensor.matmul(ph_ps[:, j * 128:(j + 1) * 128],
                                 t_qpi[:], t_throws[j][0:1, :],
                                 start=True, stop=True)
            # ph = q*theta/(2*pi).  T_q = cos(2*pi*ph) = -sin(2*pi*u) with
            # u = (ph - 1/4) - rint(ph - 1/4); the -1 is folded into cons[:,1].
            t_e = epi.tile([NQ, BL], F32)
            nc.vector.tensor_scalar(t_e[:], ph_ps[:], -0.25, None, OP.add)
            t_re = epi.tile([NQ, BL], F32)
            nc.vector.tensor_scalar(t_re[:], t_e[:], MAGIC, MAGIC,
                                    OP.add, OP.subtract)
            t_u = epi.tile([NQ, BL], F32)
            nc.vector.tensor_sub(t_u[:], t_e[:], t_re[:])
            t_outer = epi.tile([NQ, BL], F32)
            nc.scalar.activation(t_outer[:], t_u[:], AF.Sin, scale=2.0 * PI)
            t_res = epi.tile([NQ, BL], F32)
            nc.vector.tensor_mul(t_res[:], t_outer[:], t_sc[:])
            nc.sync.dma_start(out_qb[:], t_res[:])

    nc.compile()
    return nc


def _prepare_static_inputs():
    """Inputs that do not depend on x (built once)."""
    return None


def _host_inputs(x, W):
    c = _coeffs()
    negW = (W[:KMAX, :IN]).astype(np.float32)       # [58, 100]

    # f32 block: k=1..15, one-hot in 32 cols (out partitions 0..31)
    wf = np.zeros((IN, K_EXACT * 32), dtype=np.float32)
    for i in range(K_EXACT):
        sgn = np.float32(-1.0) if i == 1 else np.float32(1.0)
        wf[:, i * 32 + i] = sgn * negW[i, :]
    # bf16 block: k=16..32 one-hot in 32 cols, then k=33..58 in 26 cols
    wh = np.zeros((IN, 17 * 32 + 26 * 26), dtype=np.float32)
    for i in range(K_EXACT, 32):
        j = i - K_EXACT
        wh[:, j * 32 + i] = negW[i, :]
    for i in range(32, KMAX):
        j = i - 32
        wh[:, 17 * 32 + j * 26 + j] = negW[i, :]
    wh = wh.astype(ml_dtypes.bfloat16)

    cons = np.zeros((128, 8), dtype=np.float32)
    cons[:NQ, 0] = c
    cons[:NQ, 1] = -c * np.float32(0.01)
    cons[:NQ, 2] = np.float32(0.01)
    cons[:, 3] = np.float32(-np.pi)
    cons[:, 4] = np.float32(np.pi / 2)
    qp = (np.arange(NQ, dtype=np.float64) / (2 * np.pi)).astype(np.float32).reshape(1, NQ)
    idm = np.eye(128, dtype=np.float32)
    shared = dict(wstat_f=wf, wstat_h=wh, cons=cons, qpi=qp, ident=idm)
    maps = []
    for ci in range(N_CORES):
        xs = x[ci * BL:(ci + 1) * BL, :]            # [512, 100]
        xtc = np.ascontiguousarray(xs.T.astype(np.float32))  # [100, 512]
        maps.append(dict(shared, xt=xtc))
    return maps


def _run(x, W, trace=False, trace_kwargs=None):
    global _PROG
    if _PROG is None:
        _PROG = _build_program()
    maps = _host_inputs(x, W)
    res = run_bass_kernel_spmd(_PROG, maps, list(range(N_CORES)),
                               trace=trace, **(trace_kwargs or {}))
    out = np.zeros((B_FULL, D_OUT), dtype=np.float32)
    for ci in range(N_CORES):
        out[ci * BL:(ci + 1) * BL, :NQ] = res.results[ci]["out_qb"].T
    return out, res


def kernel(x, inner_coefficients, theta_matrix, dimension):
    x = np.asarray(x, dtype=np.float32)
    W = np.asarray(inner_coefficients, dtype=np.float32)
    out, _ = _run(x, W, trace=False)
    return out
